# revision 1
# baseline (speedup 1.0000x reference)
"""GatedGCN (NewGraphReasoner) Trainium2 kernel — 8-core edge-parallel SPMD.

Strategy:
  * Nodes sharded 12500/core; edges assigned to the core owning dst (sorted
    by dst) so segment sums are core-local (no all-reduce of node aggregates).
  * Per layer: node matmuls (A/B/D/E) from own x slice, AllGather of the
    gather-tables Ex/Bx (bf16), per-edge-chunk pipeline (gather + Ce matmul +
    sigmoid + one-hot segment matmul into PSUM), masked BN stats via
    ones-matmul, tiny AllReduce of stats, then BN-apply passes.
  * All activations stored node/edge-major; matmul lhsT tiles are produced by
    bf16 DMA-transpose loads (no PE transposes).
  * Decoder uses P=x@W1a, Q=x@W1b gather tables so edge_repr never needs a
    device transpose; dec2 is a fused multiply+reduce on DVE.
"""

import os
import sys

import numpy as np

sys.path.insert(0, "/opt/trn_rl_repo")

import ml_dtypes

H = 256
L = 4
N = 100000
E = 300000
BN_EPS = 1e-5
AGG_EPS = 1e-6
NCORES = 8
NP_ = 12500          # nodes per core
NPAD = 12544         # padded (98*128)
NT = 98              # node tiles per core
P = 128

DEBUG_TAPS = bool(int(os.environ.get("KERNEL_DEBUG_TAPS", "0")))
PROFILE = bool(int(os.environ.get("KERNEL_PROFILE", "0")))

_bf16 = ml_dtypes.bfloat16


# ----------------------------------------------------------------- host prep
def _host_prep(edge_index, aligned):
    """Sort edges by (owner, dst); build uniform chunk structure + metadata."""
    src = edge_index[0].astype(np.int64)
    dst = edge_index[1].astype(np.int64)
    owner = dst // NP_
    order = np.lexsort((dst, owner))
    src_s, dst_s, owner_s = src[order], dst[order], owner[order]

    # per (core, node-tile) edge counts
    dst_loc = dst_s - owner_s * NP_
    tile_of = dst_loc // P
    counts = np.zeros((NCORES, NT), dtype=np.int64)
    np.add.at(counts, (owner_s, tile_of), 1)
    K_t = np.maximum(1, np.ceil(counts.max(axis=0) / P).astype(np.int64))  # [NT]
    nch = int(K_t.sum())
    EC = nch * P

    # chunk slot layout: for tile t, chunks occupy rows [chunk_base[t], +K_t[t])
    chunk_base = np.concatenate([[0], np.cumsum(K_t)[:-1]])

    meta_i = np.zeros((NCORES, nch, P, 2), dtype=np.int32)
    meta_f = np.zeros((NCORES, nch, P, 2), dtype=np.float32)
    meta_f[..., 0] = -1.0  # dst_rel pad
    al_t = np.zeros((NCORES, 8, EC), dtype=_bf16)
    inv_pos = np.zeros(E, dtype=np.int64)  # padded position of each sorted edge

    core_starts = np.searchsorted(owner_s, np.arange(NCORES + 1))
    for c in range(NCORES):
        lo, hi = core_starts[c], core_starts[c + 1]
        tl, dl = tile_of[lo:hi], dst_loc[lo:hi]
        sg = src_s[lo:hi]
        # position within tile-group
        tstarts = np.searchsorted(tl, np.arange(NT + 1))
        pos_in_tile = np.arange(hi - lo) - tstarts[tl]
        slot = chunk_base[tl] * P + pos_in_tile  # flat padded slot
        r = sg // NP_
        src_pg = r * NPAD + (sg - r * NP_)
        meta_i[c].reshape(-1, 2)[slot, 0] = src_pg
        meta_i[c].reshape(-1, 2)[slot, 1] = dl
        meta_f[c].reshape(-1, 2)[slot, 0] = (dl - tl * P).astype(np.float32)
        meta_f[c].reshape(-1, 2)[slot, 1] = 1.0
        al_t[c][:, slot] = aligned[order[lo:hi]].T.astype(_bf16)
        inv_pos[lo:hi] = c * EC + slot

    # node-tile mask (last tile has 84 real nodes)
    nmask = np.zeros((NT, P, 1), dtype=np.float32)
    nmask.reshape(NT * P, 1)[:NP_] = 1.0

    perm = np.empty(E, dtype=np.int64)
    perm[order] = inv_pos  # original edge i -> padded slot
    return dict(meta_i=meta_i, meta_f=meta_f, al_t=al_t, nmask=nmask,
                nch=nch, EC=EC, K_t=K_t, perm=perm)


# ------------------------------------------------------------- device kernel
def _build_nc(nch, K_t):
    import concourse.bass as bass
    import concourse.mybir as mybir
    import concourse.tile as tile
    from concourse.tile import TileContext

    F32 = mybir.dt.float32
    BF16 = mybir.dt.bfloat16
    I32 = mybir.dt.int32
    AF = mybir.ActivationFunctionType
    ALU = mybir.AluOpType
    EC = nch * P

    nc = bass.Bass("TRN2", target_bir_lowering=False, debug=False,
                   num_devices=NCORES)

    # ---------------- I/O ----------------
    def inp(name, shape, dt=F32):
        return nc.dram_tensor(name, shape, dt, kind="ExternalInput")

    ht_b = inp("ht_b", [NPAD, H], BF16)
    al_t = inp("al_t", [8, EC], BF16)
    meta_i = inp("meta_i", [nch * P, 2], I32)
    meta_f = inp("meta_f", [nch * P, 2], F32)
    nmask = inp("nmask", [NT * P, 1])
    w_f2 = inp("w_f2", [2, P, H])          # fusion_w[256:512]
    c0_r = inp("c0_r", [1, H])             # ones@fusion_w[:256]+fusion_b
    w_ep = inp("w_ep", [8, H])             # eproj_w
    b_ep = inp("b_ep", [1, H])
    wl = inp("wl", [L, 5, 2, P, H])        # A,B,C,D,E stacked
    hb = inp("hb", [L, 1, H])              # C_b+D_b+E_b per layer
    ab = inp("ab", [L, 1, H])              # A_b
    gb = inp("gb", [L, 4, 1, H])           # gx,bx,ge,be
    w1a = inp("w1a", [2, P, H])
    w1b = inp("w1b", [2, P, H])
    w1c = inp("w1c", [8, H])
    b_d1 = inp("b_d1", [1, H])
    w2_r = inp("w2_r", [1, H])             # dec2_w row

    out = nc.dram_tensor("out", [EC, 1], F32, kind="ExternalOutput")
    taps = {}
    if DEBUG_TAPS:
        for nm in ["tap_x1", "tap_x2", "tap_x3", "tap_x4", "tap_x5"]:
            taps[nm] = nc.dram_tensor(nm, [NPAD, H], F32, kind="ExternalOutput")
        taps["tap_e1"] = nc.dram_tensor("tap_e1", [EC, H], F32,
                                        kind="ExternalOutput")

    core_ids = list(range(NCORES))

    with TileContext(nc) as tc:
        import contextlib
        ctx = contextlib.ExitStack()
        with ctx:
            wpool = ctx.enter_context(tc.tile_pool(name="wp", bufs=1))
            sb = ctx.enter_context(tc.tile_pool(name="sb", bufs=3))
            sb2 = ctx.enter_context(tc.tile_pool(name="sb2", bufs=3))
            ps = ctx.enter_context(tc.tile_pool(name="ps", bufs=3, space="PSUM"))
            psA = ctx.enter_context(tc.tile_pool(name="psA", bufs=1, space="PSUM"))
            dr = ctx.enter_context(tc.tile_pool(name="dr", bufs=1, space="DRAM"))

            # ------------- persistent DRAM state -------------
            x_f = dr.tile([NPAD, H], F32, tag="x_f")
            x_b = dr.tile([NPAD, H], BF16, tag="x_b")
            e_f = dr.tile([EC, H], F32, tag="e_f")
            e_b = dr.tile([EC, H], BF16, tag="e_b")
            eh_b = dr.tile([EC, H], BF16, tag="eh_b")
            ax_b = dr.tile([NPAD, H], BF16, tag="ax_b")
            xg_b = dr.tile([NPAD, H], BF16, tag="xg_b")      # x_agg
            dx_b = dr.tile([NPAD, H], BF16, tag="dx_b")
            exc = dr.tile([NPAD, H], BF16, tag="exc")
            bxc = dr.tile([NPAD, H], BF16, tag="bxc")
            exag_l = [dr.tile([NCORES * NPAD, H], BF16, tag=f"exag{l}",
                              name=f"exag{l}", addr_space="Shared")
                      for l in range(L)]
            bxag_l = [dr.tile([NCORES * NPAD, H], BF16, tag=f"bxag{l}",
                              name=f"bxag{l}", addr_space="Shared")
                      for l in range(L)]
            p_c = dr.tile([NPAD, H], F32, tag="p_c")
            q_c = dr.tile([NPAD, H], F32, tag="q_c")
            pag = dr.tile([NCORES * NPAD, H], F32, tag="pag",
                          addr_space="Shared")
            st_i = dr.tile([1, 4 * H], F32, tag="st_i")
            st_o_l = [dr.tile([1, 4 * H], F32, tag=f"st_o{l}",
                              name=f"st_o{l}", addr_space="Shared")
                      for l in range(L)]

            # ------------- load weights to SBUF (bf16) -------------
            def wtile(name, src, shape):
                t = wpool.tile(shape, BF16, tag=name, name=name)
                if len(shape) == 3:
                    for k in range(shape[1]):
                        nc.gpsimd.dma_start(out=t[:, k, :], in_=src[k])
                else:
                    nc.gpsimd.dma_start(out=t[:], in_=src)
                return t

            t_wf2 = wtile("t_wf2", w_f2, [P, 2, H])
            t_c0 = wtile("t_c0", c0_r[:], [1, H])
            t_wep = wtile("t_wep", w_ep[:], [8, H])
            t_bep = wtile("t_bep", b_ep[:], [1, H])
            t_wl = [[wtile(f"t_wl{l}_{j}", wl[l, j], [P, 2, H])
                     for j in range(5)] for l in range(L)]
            t_hb = [wtile(f"t_hb{l}", hb[l], [1, H]) for l in range(L)]
            t_ab = [wtile(f"t_ab{l}", ab[l], [1, H]) for l in range(L)]
            t_w1a = wtile("t_w1a", w1a, [P, 2, H])
            t_w1b = wtile("t_w1b", w1b, [P, 2, H])
            t_w1c = wtile("t_w1c", w1c[:], [8, H])
            t_bd1 = wtile("t_bd1", b_d1[:], [1, H])
            t_w2 = wtile("t_w2", w2_r[:], [1, H])

            ones1 = wpool.tile([1, P], BF16, tag="ones1", name="ones1")
            nc.gpsimd.memset(ones1[:], 1.0)
            io_i = wpool.tile([P, P], I32, tag="io_i", name="io_i")
            nc.gpsimd.iota(io_i[:], pattern=[[1, P]], base=0,
                           channel_multiplier=0)
            io_f = wpool.tile([P, P], F32, tag="io_f", name="io_f")
            nc.vector.tensor_copy(io_f[:], io_i[:])

            # broadcast helper: [1,H] bf16 row -> [128,H] bf16 tile
            def bcast_row(row_bf, name):
                pm = ps.tile([P, H], F32, tag="pm", name=f"{name}_pm")
                nc.tensor.matmul(out=pm[:], lhsT=ones1[:], rhs=row_bf[:],
                                 start=True, stop=True)
                t = sb2.tile([P, H], BF16, tag=name, name=name, bufs=1)
                nc.scalar.activation(t[:], pm[:], AF.Copy)
                return t

            t_w2bc = bcast_row(t_w2, "t_w2bc")

            # lhsT loader: DRAM node/edge-major bf16 [128 rows, 256] ->
            # two SBUF [128,128] transposed tiles
            def load_lhsT(src_rows, tag):
                tt = sb.tile([P, 2, P], BF16, tag=tag, name=tag)
                for k in range(2):
                    nc.sync.dma_start_transpose(
                        tt[:, k, :], src_rows[:, k * P:(k + 1) * P])
                return tt

            # K=1 bias accumulate into psum
            def bias_acc(pm, row_bf, stop=True):
                nc.tensor.matmul(out=pm[:], lhsT=ones1[:, :pm.shape[0]],
                                 rhs=row_bf[:], start=False, stop=stop)

            # ---------------- Phase 1: fusion + eproj ----------------
            for t in range(NT):
                r0 = t * P
                lh = load_lhsT(ht_b[r0:r0 + P, :], "lh_fu")
                pm = ps.tile([P, H], F32, tag="pm", name="pm_fu")
                for k in range(2):
                    nc.tensor.matmul(out=pm[:], lhsT=lh[:, k, :],
                                     rhs=t_wf2[:, k, :], start=(k == 0),
                                     stop=False)
                bias_acc(pm, t_c0)
                xt = sb.tile([P, H], F32, tag="xt_fu", name="xt_fu")
                nc.scalar.activation(xt[:], pm[:], AF.Relu)
                nc.sync.dma_start(out=x_f[r0:r0 + P, :], in_=xt[:])
                nc.gpsimd.dma_start(out=x_b[r0:r0 + P, :], in_=xt[:])
            if DEBUG_TAPS:
                nc.sync.dma_start(out=taps["tap_x1"][:], in_=x_f[:])

            for ch in range(nch):
                c0 = ch * P
                alt = sb.tile([8, P], BF16, tag="alt_ep", name="alt_ep")
                nc.sync.dma_start(out=alt[:], in_=al_t[:, c0:c0 + P])
                pm = ps.tile([P, H], F32, tag="pm", name="pm_ep")
                nc.tensor.matmul(out=pm[:], lhsT=alt[:], rhs=t_wep[:],
                                 start=True, stop=False)
                bias_acc(pm, t_bep)
                et = sb.tile([P, H], F32, tag="et_ep", name="et_ep")
                nc.scalar.activation(et[:], pm[:], AF.Copy)
                nc.sync.dma_start(out=e_f[c0:c0 + P, :], in_=et[:])
                nc.gpsimd.dma_start(out=e_b[c0:c0 + P, :], in_=et[:])
            if DEBUG_TAPS:
                nc.sync.dma_start(out=taps["tap_e1"][:], in_=e_f[:])

            # ---------------- Layers ----------------
            for l in range(L):
                last = (l == L - 1)
                wA, wB, wC, wD, wE = t_wl[l]

                # (a) node matmuls from own x slice
                for t in range(NT):
                    r0 = t * P
                    lh = load_lhsT(x_b[r0:r0 + P, :], "lh_nm")
                    for j, (wj, dst_t) in enumerate(
                            [(wA, None), (wB, bxc), (wD, dx_b), (wE, exc)]):
                        pm = ps.tile([P, H], F32, tag="pm",
                                      name=f"pm_n{j}")
                        for k in range(2):
                            nc.tensor.matmul(out=pm[:], lhsT=lh[:, k, :],
                                             rhs=wj[:, k, :], start=(k == 0),
                                             stop=(j != 0 and k == 1))
                        if j == 0:
                            bias_acc(pm, t_ab[l])
                        ot = sb.tile([P, H], BF16, tag=f"ot_n{j}",
                                     name=f"ot_n{j}")
                        nc.scalar.activation(ot[:], pm[:], AF.Copy)
                        tgt = ax_b if j == 0 else dst_t
                        nc.sync.dma_start(out=tgt[r0:r0 + P, :], in_=ot[:])

                # (b) AllGather Ex, Bx
                exag, bxag = exag_l[l], bxag_l[l]
                nc.gpsimd.collective_compute(
                    "AllGather", ALU.bypass, replica_groups=[core_ids],
                    ins=[exc.opt()], outs=[exag.opt()])
                nc.gpsimd.collective_compute(
                    "AllGather", ALU.bypass, replica_groups=[core_ids],
                    ins=[bxc.opt()], outs=[bxag.opt()])

                # (c) edge phase + x_agg + stats
                st_x = psA.tile([1, 2 * H], F32, tag="st_x", name="st_x", bufs=1)
                st_e = psA.tile([1, 2 * H], F32, tag="st_e", name="st_e", bufs=1)
                ch = 0
                for t in range(NT):
                    pm_seg = psA.tile([P, 2 * H], F32, tag="pm_seg",
                                      name="pm_seg", bufs=2)
                    for k in range(int(K_t[t])):
                        c0 = ch * P
                        mi = sb.tile([P, 2], I32, tag="mi", name="mi")
                        nc.sync.dma_start(out=mi[:], in_=meta_i[c0:c0 + P, :])
                        mf = sb.tile([P, 2], F32, tag="mf", name="mf")
                        nc.sync.dma_start(out=mf[:], in_=meta_f[c0:c0 + P, :])
                        # gathers: EDx = Ex[src] + Dx[dst]; Bxg = Bx[src]
                        edx = sb.tile([P, H], BF16, tag="edx", name="edx")
                        nc.gpsimd.indirect_dma_start(
                            out=edx[:], out_offset=None, in_=exag[:],
                            in_offset=bass.IndirectOffsetOnAxis(
                                ap=mi[:, 0:1], axis=0))
                        nc.gpsimd.indirect_dma_start(
                            out=edx[:], out_offset=None, in_=dx_b[:],
                            in_offset=bass.IndirectOffsetOnAxis(
                                ap=mi[:, 1:2], axis=0),
                            compute_op=ALU.add)
                        bxg = sb.tile([P, H], BF16, tag="bxg", name="bxg")
                        nc.gpsimd.indirect_dma_start(
                            out=bxg[:], out_offset=None, in_=bxag[:],
                            in_offset=bass.IndirectOffsetOnAxis(
                                ap=mi[:, 0:1], axis=0))
                        # Ce
                        lh = load_lhsT(e_b[c0:c0 + P, :], "lh_ce")
                        pm = ps.tile([P, H], F32, tag="pm", name="pm_ce")
                        for k2 in range(2):
                            nc.tensor.matmul(out=pm[:], lhsT=lh[:, k2, :],
                                             rhs=wC[:, k2, :],
                                             start=(k2 == 0), stop=False)
                        bias_acc(pm, t_hb[l])
                        # e_hat (+ stats tile), sigma, sb
                        stt = sb.tile([P, 2 * H], BF16, tag="stt", name="stt")
                        nc.vector.tensor_tensor(out=stt[:, 0:H], in0=pm[:],
                                                in1=edx[:], op=ALU.add)
                        nc.sync.dma_start(out=eh_b[c0:c0 + P, :],
                                          in_=stt[:, 0:H])
                        seg = sb.tile([P, 2 * H], BF16, tag="seg", name="seg")
                        nc.scalar.activation(seg[:, H:2 * H], stt[:, 0:H],
                                             AF.Sigmoid)
                        nc.vector.tensor_tensor(out=seg[:, 0:H],
                                                in0=seg[:, H:2 * H],
                                                in1=bxg[:], op=ALU.mult)
                        # one-hot + seg matmul
                        sel = sb.tile([P, P], BF16, tag="sel", name="sel")
                        nc.vector.tensor_tensor(
                            out=sel[:], in0=mf[:, 0:1].to_broadcast([P, P]),
                            in1=io_f[:], op=ALU.is_equal)
                        nc.tensor.matmul(out=pm_seg[:], lhsT=sel[:],
                                         rhs=seg[:], start=(k == 0),
                                         stop=(k == int(K_t[t]) - 1))
                        # e-hat stats (masked)
                        if not last:
                            nc.scalar.activation(stt[:, H:2 * H],
                                                 stt[:, 0:H], AF.Square)
                            msk = sb.tile([P, 1], BF16, tag="msk", name="msk")
                            nc.vector.tensor_copy(msk[:], mf[:, 1:2])
                            nc.tensor.matmul(out=st_e[:], lhsT=msk[:],
                                             rhs=stt[:], start=(ch == 0),
                                             stop=(ch == nch - 1))
                        ch += 1
                    # x_agg for tile t
                    r0 = t * P
                    dn = sb.tile([P, H], F32, tag="dn", name="dn")
                    nc.vector.tensor_scalar_add(dn[:], pm_seg[:, H:2 * H], AGG_EPS)
                    rc = sb.tile([P, H], F32, tag="rc", name="rc")
                    nc.vector.reciprocal(rc[:], dn[:])
                    agg = sb.tile([P, H], F32, tag="agg", name="agg")
                    nc.vector.tensor_tensor(out=agg[:], in0=pm_seg[:, 0:H],
                                            in1=rc[:], op=ALU.mult)
                    axt = sb.tile([P, H], BF16, tag="axt", name="axt")
                    nc.sync.dma_start(out=axt[:], in_=ax_b[r0:r0 + P, :])
                    stx = sb.tile([P, 2 * H], BF16, tag="stx", name="stx")
                    nc.vector.tensor_tensor(out=stx[:, 0:H], in0=agg[:],
                                            in1=axt[:], op=ALU.add)
                    nc.sync.dma_start(out=xg_b[r0:r0 + P, :], in_=stx[:, 0:H])
                    nc.scalar.activation(stx[:, H:2 * H], stx[:, 0:H],
                                         AF.Square)
                    nm = sb.tile([P, 1], BF16, tag="nm", name="nm")
                    nmf = sb.tile([P, 1], F32, tag="nmf", name="nmf")
                    nc.sync.dma_start(out=nmf[:], in_=nmask[r0:r0 + P, :])
                    nc.vector.tensor_copy(nm[:], nmf[:])
                    nc.tensor.matmul(out=st_x[:], lhsT=nm[:], rhs=stx[:],
                                     start=(t == 0), stop=(t == NT - 1))

                # (d) stats -> DRAM -> AllReduce
                stc = sb.tile([1, 4 * H], F32, tag="stc", name="stc", bufs=1)
                nc.vector.tensor_copy(stc[:, 0:2 * H], st_x[:])
                if not last:
                    nc.vector.tensor_copy(stc[:, 2 * H:4 * H], st_e[:])
                else:
                    nc.gpsimd.memset(stc[:, 2 * H:4 * H], 0.0)
                nc.sync.dma_start(out=st_i[:], in_=stc[:])
                st_o = st_o_l[l]
                nc.gpsimd.collective_compute(
                    "AllReduce", ALU.add, replica_groups=[core_ids],
                    ins=[st_i.opt()], outs=[st_o.opt()])

                # (e) s,t from stats; broadcast tiles
                stg = sb.tile([1, 4 * H], F32, tag="stg", name="stg", bufs=1)
                nc.sync.dma_start(out=stg[:], in_=st_o[:])

                def bn_st(sl, cnt, g_ap, b_ap, nm_):
                    mu = sb2.tile([1, H], F32, tag=f"mu{nm_}", name=f"mu{nm_}")
                    nc.scalar.mul(mu[:], stg[:, sl:sl + H], 1.0 / cnt)
                    m2 = sb2.tile([1, H], F32, tag=f"m2{nm_}", name=f"m2{nm_}")
                    nc.scalar.mul(m2[:], stg[:, sl + H:sl + 2 * H], 1.0 / cnt)
                    mu2 = sb2.tile([1, H], F32, tag=f"mu2{nm_}",
                                   name=f"mu2{nm_}")
                    nc.vector.tensor_tensor(out=mu2[:], in0=mu[:], in1=mu[:],
                                            op=ALU.mult)
                    var = sb2.tile([1, H], F32, tag=f"var{nm_}",
                                   name=f"var{nm_}")
                    nc.vector.tensor_tensor(out=var[:], in0=m2[:], in1=mu2[:],
                                            op=ALU.subtract)
                    nc.vector.tensor_scalar_add(var[:], var[:], BN_EPS)
                    sd = sb2.tile([1, H], F32, tag=f"sd{nm_}", name=f"sd{nm_}")
                    nc.scalar.activation(sd[:], var[:], AF.Sqrt)
                    rs = sb2.tile([1, H], F32, tag=f"rs{nm_}", name=f"rs{nm_}")
                    nc.vector.reciprocal(rs[:], sd[:])
                    gg = sb2.tile([1, H], F32, tag=f"gg{nm_}", name=f"gg{nm_}")
                    nc.sync.dma_start(out=gg[:], in_=g_ap)
                    bb = sb2.tile([1, H], F32, tag=f"bb{nm_}", name=f"bb{nm_}")
                    nc.sync.dma_start(out=bb[:], in_=b_ap)
                    s_ = sb2.tile([1, H], BF16, tag=f"s_{nm_}", name=f"s_{nm_}")
                    nc.vector.tensor_tensor(out=s_[:], in0=gg[:], in1=rs[:],
                                            op=ALU.mult)
                    mus = sb2.tile([1, H], F32, tag=f"mus{nm_}",
                                   name=f"mus{nm_}")
                    nc.vector.tensor_tensor(out=mus[:], in0=mu[:], in1=s_[:],
                                            op=ALU.mult)
                    t_ = sb2.tile([1, H], BF16, tag=f"t_{nm_}", name=f"t_{nm_}")
                    nc.vector.tensor_tensor(out=t_[:], in0=bb[:], in1=mus[:],
                                            op=ALU.subtract)
                    return bcast_row(s_, f"sbc{nm_}"), bcast_row(t_, f"tbc{nm_}")

                sx_bc, tx_bc = bn_st(0, N, gb[l, 0], gb[l, 1], "x")
                if not last:
                    se_bc, te_bc = bn_st(2 * H, E, gb[l, 2], gb[l, 3], "e")

                # (f) pass-2 x: x_new = x_in + relu(bn(x_agg))
                for t in range(NT):
                    r0 = t * P
                    xa = sb.tile([P, H], BF16, tag="xa", name="xa")
                    nc.sync.dma_start(out=xa[:], in_=xg_b[r0:r0 + P, :])
                    t1 = sb.tile([P, H], F32, tag="t1x", name="t1x")
                    nc.vector.tensor_tensor(out=t1[:], in0=xa[:], in1=sx_bc[:],
                                            op=ALU.mult)
                    t2 = sb.tile([P, H], F32, tag="t2x", name="t2x")
                    nc.vector.tensor_tensor(out=t2[:], in0=t1[:], in1=tx_bc[:],
                                            op=ALU.add)
                    nc.scalar.activation(t2[:], t2[:], AF.Relu)
                    xi = sb.tile([P, H], F32, tag="xi", name="xi")
                    nc.sync.dma_start(out=xi[:], in_=x_f[r0:r0 + P, :])
                    xn = sb.tile([P, H], F32, tag="xn", name="xn")
                    nc.vector.tensor_tensor(out=xn[:], in0=xi[:], in1=t2[:],
                                            op=ALU.add)
                    nc.sync.dma_start(out=x_f[r0:r0 + P, :], in_=xn[:])
                    nc.gpsimd.dma_start(out=x_b[r0:r0 + P, :], in_=xn[:])
                if DEBUG_TAPS:
                    nc.sync.dma_start(out=taps[f"tap_x{l + 2}"][:], in_=x_f[:])

                # (g) pass-2 e
                if not last:
                    for ch2 in range(nch):
                        c0 = ch2 * P
                        ea = sb.tile([P, H], BF16, tag="ea", name="ea")
                        nc.sync.dma_start(out=ea[:], in_=eh_b[c0:c0 + P, :])
                        t1 = sb.tile([P, H], F32, tag="t1e", name="t1e")
                        nc.vector.tensor_tensor(out=t1[:], in0=ea[:],
                                                in1=se_bc[:], op=ALU.mult)
                        t2 = sb.tile([P, H], F32, tag="t2e", name="t2e")
                        nc.vector.tensor_tensor(out=t2[:], in0=t1[:],
                                                in1=te_bc[:], op=ALU.add)
                        nc.scalar.activation(t2[:], t2[:], AF.Relu)
                        ei = sb.tile([P, H], F32, tag="ei", name="ei")
                        nc.sync.dma_start(out=ei[:], in_=e_f[c0:c0 + P, :])
                        en = sb.tile([P, H], F32, tag="en", name="en")
                        nc.vector.tensor_tensor(out=en[:], in0=ei[:],
                                                in1=t2[:], op=ALU.add)
                        nc.sync.dma_start(out=e_f[c0:c0 + P, :], in_=en[:])
                        nc.gpsimd.dma_start(out=e_b[c0:c0 + P, :], in_=en[:])

            # ---------------- Decoder ----------------
            for t in range(NT):
                r0 = t * P
                lh = load_lhsT(x_b[r0:r0 + P, :], "lh_pq")
                for wj, dst_t in [(t_w1a, p_c), (t_w1b, q_c)]:
                    pm = ps.tile([P, H], F32, tag="pm", name="pm_pq")
                    for k in range(2):
                        nc.tensor.matmul(out=pm[:], lhsT=lh[:, k, :],
                                         rhs=wj[:, k, :], start=(k == 0),
                                         stop=(k == 1))
                    ot = sb.tile([P, H], F32, tag="ot_pq", name="ot_pq")
                    nc.scalar.activation(ot[:], pm[:], AF.Copy)
                    nc.sync.dma_start(out=dst_t[r0:r0 + P, :], in_=ot[:])
            nc.gpsimd.collective_compute(
                "AllGather", ALU.bypass, replica_groups=[core_ids],
                ins=[p_c.opt()], outs=[pag.opt()])

            for ch2 in range(nch):
                c0 = ch2 * P
                mi = sb.tile([P, 2], I32, tag="mi_d", name="mi_d")
                nc.sync.dma_start(out=mi[:], in_=meta_i[c0:c0 + P, :])
                pgt = sb.tile([P, H], F32, tag="pgt", name="pgt")
                nc.gpsimd.indirect_dma_start(
                    out=pgt[:], out_offset=None, in_=pag[:],
                    in_offset=bass.IndirectOffsetOnAxis(ap=mi[:, 0:1], axis=0))
                qgt = sb.tile([P, H], F32, tag="qgt", name="qgt")
                nc.gpsimd.indirect_dma_start(
                    out=qgt[:], out_offset=None, in_=q_c[:],
                    in_offset=bass.IndirectOffsetOnAxis(ap=mi[:, 1:2], axis=0))
                alt = sb.tile([8, P], BF16, tag="alt_d", name="alt_d")
                nc.sync.dma_start(out=alt[:], in_=al_t[:, c0:c0 + P])
                pm = ps.tile([P, H], F32, tag="pm", name="pm_d")
                nc.tensor.matmul(out=pm[:], lhsT=alt[:], rhs=t_w1c[:],
                                 start=True, stop=False)
                bias_acc(pm, t_bd1)
                h1 = sb.tile([P, H], F32, tag="h1", name="h1")
                nc.vector.tensor_tensor(out=h1[:], in0=pgt[:], in1=qgt[:],
                                        op=ALU.add)
                h2 = sb.tile([P, H], F32, tag="h2", name="h2")
                nc.vector.tensor_tensor(out=h2[:], in0=h1[:], in1=pm[:],
                                        op=ALU.add)
                nc.scalar.activation(h2[:], h2[:], AF.Relu)
                ov = sb.tile([P, H], F32, tag="ov", name="ov")
                oa = sb.tile([P, 1], F32, tag="oa", name="oa")
                nc.vector.tensor_tensor(out=ov[:], in0=h2[:],
                                        in1=t_w2bc[:], op=ALU.mult)
                nc.vector.tensor_reduce(out=oa[:], in_=ov[:], op=ALU.add,
                                        axis=mybir.AxisListType.X)
                nc.sync.dma_start(out=out[c0:c0 + P, :], in_=oa[:])

    _split_excess_waits(nc, mybir)
    return nc


def _split_excess_waits(nc, mybir, max_waits=1):
    """walrus in this env accepts max 1 sem wait per instruction: spill
    extras onto same-engine nops placed before the instruction."""
    for f in nc.m.functions:
        for bb in f.blocks:
            insts = list(bb.instructions)
            out_l = []
            for inst in insts:
                si = inst.sync_info
                waits = list(si.on_wait) if (si and si.on_wait) else []
                if len(waits) > max_waits:
                    extra = waits[:-max_waits]
                    keep = waits[-max_waits:]
                    for i in range(0, len(extra), max_waits):
                        nop = mybir.InstNoOp(
                            name=nc.get_next_instruction_name(),
                            engine=inst.engine, ins=[], outs=[],
                            sync_info=mybir.SyncInfo(
                                on_wait=extra[i:i + max_waits], on_update=[]))
                        nc.register_instruction(nop)
                        out_l.append(nop)
                    del si.on_wait[:]
                    si.on_wait.extend(keep)
                out_l.append(inst)
            if len(out_l) != len(insts):
                bb.instructions = out_l


# ----------------------------------------------------------------- wrapper
_CACHE = {}


def kernel(**inputs):
    edge_index = np.asarray(inputs["edge_index_new"])
    aligned = np.asarray(inputs["aligned_features"], dtype=np.float32)
    h_old = np.asarray(inputs["h_nodes_old"], dtype=np.float32)
    assert int(inputs["num_nodes"]) == N

    prep = _host_prep(edge_index, aligned)
    nch, EC = prep["nch"], prep["EC"]

    fw = np.asarray(inputs["fusion_w"], np.float32)
    fb = np.asarray(inputs["fusion_b"], np.float32)
    c0 = fw[:H].sum(axis=0) + fb

    wl = np.stack([np.stack([np.asarray(inputs[f"{nm}_w"], np.float32)[l]
                             for nm in "ABCDE"])
                   for l in range(L)])                       # [L,5,256,256]
    hb = np.stack([(np.asarray(inputs["C_b"], np.float32)[l]
                    + np.asarray(inputs["D_b"], np.float32)[l]
                    + np.asarray(inputs["E_b"], np.float32)[l])[None]
                   for l in range(L)])
    ab = np.stack([np.asarray(inputs["A_b"], np.float32)[l][None]
                   for l in range(L)])
    gb = np.stack([np.stack([np.asarray(inputs["bn_x_g"], np.float32)[l][None],
                             np.asarray(inputs["bn_x_b"], np.float32)[l][None],
                             np.asarray(inputs["bn_e_g"], np.float32)[l][None],
                             np.asarray(inputs["bn_e_b"], np.float32)[l][None]])
                   for l in range(L)])
    d1 = np.asarray(inputs["dec1_w"], np.float32)            # [520, 256]
    d2 = np.asarray(inputs["dec2_w"], np.float32)            # [256, 1]

    key = (nch,) + tuple(prep["K_t"])
    if key not in _CACHE:
        _CACHE[key] = _build_nc(nch, prep["K_t"])
    nc = _CACHE[key]

    shared = {
        "w_f2": fw[H:].reshape(2, P, H), "c0_r": c0[None],
        "w_ep": np.asarray(inputs["eproj_w"], np.float32),
        "b_ep": np.asarray(inputs["eproj_b"], np.float32)[None],
        "wl": wl.reshape(L, 5, 2, P, H), "hb": hb, "ab": ab, "gb": gb,
        "w1a": d1[:H].reshape(2, P, H), "w1b": d1[H:2 * H].reshape(2, P, H),
        "w1c": d1[2 * H:],
        "b_d1": np.asarray(inputs["dec1_b"], np.float32)[None],
        "w2_r": d2[:, 0][None],
        "nmask": prep["nmask"].reshape(NT * P, 1),
    }
    in_maps = []
    for c in range(NCORES):
        lo = c * NP_
        hpad = np.zeros((NPAD, H), dtype=_bf16)
        hpad[:NP_] = h_old[lo:lo + NP_].astype(_bf16)
        m = dict(shared)
        m["ht_b"] = hpad
        m["al_t"] = prep["al_t"][c]
        m["meta_i"] = prep["meta_i"][c].reshape(nch * P, 2)
        m["meta_f"] = prep["meta_f"][c].reshape(nch * P, 2)
        in_maps.append(m)

    from concourse.bass_utils import run_bass_kernel_spmd
    res = run_bass_kernel_spmd(nc, in_maps, list(range(NCORES)),
                               trace=PROFILE)
    if PROFILE and res.exec_time_ns is not None:
        print(f"HW exec time: {res.exec_time_ns} ns")

    allout = np.concatenate([res.results[c]["out"][:, 0]
                             for c in range(NCORES)])
    b2 = float(np.asarray(inputs["dec2_b"], np.float32).ravel()[0])
    flow = (allout[prep["perm"]] + b2).astype(np.float32)[:, None]
    if DEBUG_TAPS:
        kernel.taps = [
            {k: v for k, v in r.items() if k.startswith("tap")}
            for r in res.results]
        kernel.prep = prep
    return flow

